# revision 28
# baseline (speedup 1.0000x reference)
"""Trainium2 Bass kernel for nn_AttentionBlock_78400333021395.

AttentionBlock: x -> x + ls1*MHSA(BN(x)) -> + ls2*ConvFFN(BN(.))
  B=64, C=512, H=W=14 (N=196 tokens), 16 heads x 32 dim, FFN hidden 2048,
  depthwise 7x7 conv, inference BN, layer-scale 1e-5.

Sharding: data-parallel over batch, 8 batches per core on 8 NeuronCores.
All BN affines / layer scales / qk scale are folded into matmul weights and
biases on the host. Matmuls run in bf16 (residual path stays f32; all
approximation error is scaled by layer_scale=1e-5 so precision headroom is
large).

Structure per core: attention runs on batch PAIRS (392-col matmul streams
for Q/K/proj), with both batches' V+S^T emitted before either batch's A@V
so the softmax exp (ScalarE) overlaps PE work, and all four A@V psum
groups emitted before the denominator-broadcast matmuls so their
reciprocal chains never head-of-line-block the PE queue. The depthwise
7x7 conv runs entirely on PE as 16 concurrent 32x32 diagonal-block
matmuls per tap (4 row x 4 col tile_position groups; Latin-square psum
placement jj=(rb+cc)%4 keeps concurrent row-tiles in distinct banks; the
conv-output channel permutation is absorbed into fc1's host-side weight
layout), accumulating all 49 taps in psum.
"""

import numpy as np
import ml_dtypes

import concourse.bass as bass
import concourse.tile as tile
from concourse import bacc, mybir
from concourse.bass_utils import run_bass_kernel_spmd

# ---- problem constants (hardcoded per harness contract) ----
B, C, H, W = 64, 512, 14, 14
N = H * W  # 196
NH, D = 16, 32
HID = 2048
EPS = 1e-5
NCORES = 8
BPC = B // NCORES  # 8 batches per core
TOK = BPC * N      # 1568 tokens per core
P = 128
CCH = C // P       # 4 channel chunks
HCH = HID // P     # 16 hidden chunks
MCHUNKS = [(0, 98), (98, 98)]        # token chunks within a batch (196 = 98+98)
PADW = 20                            # padded spatial width (14 + 2*3)
HB = BPC // 2                        # 4 batches per FFN half
NR2 = HB * PADW                      # 80 merged rows
NPAD2 = HB * PADW * PADW + 6 * PADW  # padded free size + dy overhang

# depthwise conv: all 49 taps on PE as 16 concurrent 32x32 diag-block
# matmuls (4 row-groups x 4 col-groups; Latin square jj=(rb+cc)%4 keeps
# concurrent row-tiles in distinct psum banks). Channel permutation of
# the conv output is absorbed into fc1 weights on the host.
NTAPS = 49

F32 = mybir.dt.float32
BF16 = mybir.dt.bfloat16
F8 = mybir.dt.float8e4
AF = mybir.ActivationFunctionType
ALU = mybir.AluOpType

_CACHE = {}
DEBUG = False
ATTN_ONLY = False
ATTN_BATCHES = 8
REPS = 1


def _build_nc():
    nc = bacc.Bacc("TRN2", target_bir_lowering=False, debug=False,
                   num_devices=NCORES)

    xin = nc.dram_tensor("xin", [CCH, P, BPC, N], F32, kind="ExternalInput")
    qkvw = nc.dram_tensor("qkvw", [CCH, P, 3 * C], BF16, kind="ExternalInput")
    qb = nc.dram_tensor("qb", [P, CCH], F32, kind="ExternalInput")
    projw = nc.dram_tensor("projw", [CCH, P, C], BF16, kind="ExternalInput")
    projb = nc.dram_tensor("projb", [P, CCH], F32, kind="ExternalInput")
    fc1w = nc.dram_tensor("fc1w", [CCH, P, HID], BF16, kind="ExternalInput")
    fc1b = nc.dram_tensor("fc1b", [P, HCH], F32, kind="ExternalInput")
    fc2w = nc.dram_tensor("fc2w", [HCH, P, C], F8, kind="ExternalInput")
    fc2b = nc.dram_tensor("fc2b", [P, CCH], F32, kind="ExternalInput")
    s1d = nc.dram_tensor("s1d", [P, CCH], F32, kind="ExternalInput")
    s2d = nc.dram_tensor("s2d", [P, CCH], F32, kind="ExternalInput")
    t1d = nc.dram_tensor("t1d", [P, CCH], F32, kind="ExternalInput")
    dwdiag = nc.dram_tensor("dwdiag", [P, NTAPS * CCH * 32], BF16,
                            kind="ExternalInput")
    selw = nc.dram_tensor("selw", [2, P], BF16, kind="ExternalInput")
    out = nc.dram_tensor("out", [CCH, P, BPC, N], F32, kind="ExternalOutput")
    dbg = {}
    if DEBUG:
        dbg["q0"] = nc.dram_tensor("dbg_q0", [P, 2 * N], F32, kind="ExternalOutput")
        dbg["k0"] = nc.dram_tensor("dbg_k0", [P, 2 * N], F32, kind="ExternalOutput")
        dbg["pt0"] = nc.dram_tensor("dbg_pt0", [P, 2, 2 * N], F32, kind="ExternalOutput")
        dbg["av0"] = nc.dram_tensor("dbg_av0", [P, 2 * N], F32, kind="ExternalOutput")
        dbg["rec0"] = nc.dram_tensor("dbg_rec0", [2, 2 * N], F32, kind="ExternalOutput")
        dbg["ot0"] = nc.dram_tensor("dbg_ot0", [P, 2 * N], F32, kind="ExternalOutput")
        dbg["x2"] = nc.dram_tensor("dbg_x2", [P, TOK], F32, kind="ExternalOutput")

    with tile.TileContext(nc) as tc:
        for _rep in range(REPS):
            _emit(nc, tc, xin, qkvw, qb, projw, projb, fc1w, fc1b, fc2w,
                  fc2b, s1d, s2d, t1d, dwdiag, selw, out, dbg)
    _strip_redundant_ldweights(nc)
    nc.compile()
    return nc


def _strip_redundant_ldweights(nc):
    """Drop an InstLdweights when the previous LDW on the PE queue loaded
    the exact same weights AP into the same tile and exactly one
    InstMatmult ran in between (the si=1 conv matmul reuses the si=0
    weights still resident in its 32x32 tile). Only sync-free LDWs are
    dropped, so all semaphore waits/updates are preserved. Instructions
    of other engines interleaved in the block don't touch PE weights."""
    removed = 0
    for f in nc.m.functions:
        for blk in f.blocks:
            insts = blk.instructions
            out_insts = []
            last_ldw = None       # (weights-AP str, tile_position str)
            mm_since_ldw = 0      # matmuls since the last kept LDW
            for i in insts:
                if isinstance(i, mybir.InstLdweights):
                    key = (str(i.ins[0]), str(i.tile_position))
                    si = i.sync_info
                    syncfree = si is None or (not si.on_wait
                                              and not si.on_update)
                    if key == last_ldw and mm_since_ldw == 1 and syncfree:
                        removed += 1
                        continue
                    last_ldw = key
                    mm_since_ldw = 0
                elif isinstance(i, mybir.InstMatmult):
                    mm_since_ldw += 1
                out_insts.append(i)
            insts[:] = out_insts
    return removed


def _emit(nc, tc, xin, qkvw, qb, projw, projb, fc1w, fc1b, fc2w, fc2b,
          s1d, s2d, t1d, dwdiag, selw, out, dbg={}):
    from contextlib import ExitStack
    ctx = ExitStack()
    wp = ctx.enter_context(tc.tile_pool(name="wp", bufs=1))
    xbp = ctx.enter_context(tc.tile_pool(name="xbp", bufs=2))
    x2p = ctx.enter_context(tc.tile_pool(name="x2p", bufs=1))
    padp = ctx.enter_context(tc.tile_pool(name="padp", bufs=1))
    qkp = ctx.enter_context(tc.tile_pool(name="qkp", bufs=4))
    vp = ctx.enter_context(tc.tile_pool(name="vp", bufs=1))
    ptp = ctx.enter_context(tc.tile_pool(name="ptp", bufs=16))
    otp = ctx.enter_context(tc.tile_pool(name="otp", bufs=8))
    rcp = ctx.enter_context(tc.tile_pool(name="rcp", bufs=3))
    oup = ctx.enter_context(tc.tile_pool(name="oup", bufs=8))
    rbp = ctx.enter_context(tc.tile_pool(name="rbp", bufs=8))
    xrp = ctx.enter_context(tc.tile_pool(name="xrp", bufs=2))
    cvp = ctx.enter_context(tc.tile_pool(name="cvp", bufs=8))
    gep = ctx.enter_context(tc.tile_pool(name="gep", bufs=1))
    outp = ctx.enter_context(tc.tile_pool(name="outp", bufs=3))
    psA = ctx.enter_context(tc.tile_pool(name="psA", bufs=4, space="PSUM"))
    ps2 = ctx.enter_context(tc.tile_pool(name="ps2", bufs=2, space="PSUM"))

    # ---- pair-0 x first (gates the first qkv matmul), then attention
    # weights, then FFN weights (not needed until ~half-way through) ----
    def load_pair_x(q):
        b0 = 2 * q
        xrs, xrbs = [], []
        for cc in range(CCH):
            xr = xrp.tile([P, 2 * N], F32, tag=f"xr{cc}", name=f"xr{q}_{cc}")
            nc.sync.dma_start(
                xr.rearrange("p (b n) -> p b n", b=2)[:],
                xin[cc, :, b0:b0 + 2, :])
            xrs.append(xr)
        for cc in range(CCH):
            xb = xbp.tile([P, 2 * N], BF16, tag=f"xrb{cc}", name=f"xrb{q}_{cc}")
            nc.vector.tensor_copy(xb[:], xrs[cc][:])
            xrbs.append(xb)
        return xrs, xrbs

    x0 = load_pair_x(0)

    def ldvec(dram, shape, tag):
        t = wp.tile(shape, F32, tag=tag, name=tag)
        nc.sync.dma_start(t[:], dram[:])
        return t

    qkv_sb, pw_sb = [], []
    for kt in range(CCH):
        t = wp.tile([P, 3 * C], BF16, tag=f"qkvw{kt}", name=f"qkvw{kt}")
        nc.sync.dma_start(t[:], qkvw[kt])
        qkv_sb.append(t)
    qb_sb = ldvec(qb, [P, CCH], "qb")
    sel_sb = wp.tile([2, P], BF16, tag="sel")
    nc.sync.dma_start(sel_sb[:], selw[:])
    for kt in range(CCH):
        t = wp.tile([P, C], BF16, tag=f"projw{kt}", name=f"projw{kt}")
        nc.sync.dma_start(t[:], projw[kt])
        pw_sb.append(t)
    pb_sb = ldvec(projb, [P, CCH], "pb")
    s1_sb = ldvec(s1d, [P, CCH], "s1")
    s2_sb = ldvec(s2d, [P, CCH], "s2")
    t1_sb = ldvec(t1d, [P, CCH], "t1")
    dwd_sb = wp.tile([P, NTAPS * CCH * 32], BF16, tag="dwd")
    nc.sync.dma_start(dwd_sb[:], dwdiag[:])
    f1_sb, f2_sb = [], []
    for kt in range(CCH):
        t = wp.tile([P, HID], BF16, tag=f"fc1w{kt}", name=f"fc1w{kt}")
        nc.sync.dma_start(t[:], fc1w[kt])
        f1_sb.append(t)
    f1b_sb = ldvec(fc1b, [P, HCH], "f1b")
    for kt in range(HCH):
        t = wp.tile([P, C], F8, tag=f"fc2w{kt}", name=f"fc2w{kt}")
        nc.sync.dma_start(t[:], fc2w[kt])
        f2_sb.append(t)
    f2b_sb = ldvec(fc2b, [P, CCH], "f2b")

    x2 = [x2p.tile([P, TOK], F32, tag=f"x2_{cc}", name=f"x2_{cc}") for cc in range(CCH)]

    # persistent V tiles: the zeros/ones scaffold (denominator trick) is
    # written once; only the v-data regions are overwritten per batch
    vts = [vp.tile([P, NH * 68], BF16, tag=f"vt{i}", name=f"vt{i}")
           for i in range(4)]
    for vt in vts:
        nc.gpsimd.memset(vt[:], 0.0)
        nc.gpsimd.memset(vt[:, 64:64 + 136 * 7 + 1:136], 1.0)
        nc.gpsimd.memset(vt[:, 133:133 + 136 * 7 + 1:136], 1.0)

    # persistent padded conv-input tiles (borders stay zero; interior
    # rewritten per half)
    xps = [padp.tile([P, NPAD2], BF16, tag=f"xpad{cc}", name=f"xpad{cc}")
           for cc in range(CCH)]
    for xp in xps:
        nc.gpsimd.memset(xp[:], 0.0)

    # ================= attention (per batch pair) =================
    # V blocks of 68 cols per head h: data v(32) at rows 0:32 (h even) /
    # 32:64 (h odd), ones-column at 64 + h%4. A@V packs the pr-pair
    # (pr=2q+j) along psum free cols (j*N); denominators land at psum
    # rows 64+2j+e in the j-th col half.
    def s1_qk(st):
        """Q/K for pair q (both batches, 392-col streams)."""
        xrb = st["xrb"]
        Q, K = [], []
        for g in range(CCH):
            psq = psA.tile([P, 512], F32, tag="ps")
            for kt in range(CCH):
                nc.tensor.matmul(psq[:, :2 * N],
                                 qkv_sb[kt][:, g * P:(g + 1) * P],
                                 xrb[kt][:],
                                 start=(kt == 0), stop=(kt == CCH - 1))
            psk = psA.tile([P, 512], F32, tag="ps")
            for kt in range(CCH):
                nc.tensor.matmul(psk[:, :2 * N],
                                 qkv_sb[kt][:, C + g * P:C + (g + 1) * P],
                                 xrb[kt][:],
                                 start=(kt == 0), stop=(kt == CCH - 1))
            qt = qkp.tile([P, 2 * N], BF16, tag="q")
            nc.vector.tensor_scalar_add(qt[:], psq[:, :2 * N],
                                        qb_sb[:, g:g + 1])
            Q.append(qt)
            kt_ = qkp.tile([P, 2 * N], BF16, tag="k")
            nc.vector.tensor_copy(kt_[:], psk[:, :2 * N])
            K.append(kt_)
        st["Q"], st["K"] = Q, K
        st["Vs"], st["PTs"] = {}, {}

    def s1_v(st, ib):
        """V for batch ib of pair q (token-major into the vt scaffold)."""
        b = st["b0"] + ib
        bo = ib * N
        xrb = st["xrb"]
        V = []
        for mc, (moff, mlen) in enumerate(MCHUNKS):
            psv = psA.tile([P, 512], F32, tag="ps")
            for kt in range(CCH):
                nc.tensor.matmul(psv[:mlen, :],
                                 xrb[kt][:, bo + moff:bo + moff + mlen],
                                 qkv_sb[kt][:, 2 * C:3 * C],
                                 start=(kt == 0), stop=(kt == CCH - 1))
            vt = vts[(b % 2) * 2 + mc]
            pv = psv[:mlen].rearrange("p (h d) -> p h d", d=32)
            vv = vt[:mlen].rearrange("p (q e) -> p q e", e=136)
            nc.vector.tensor_copy(vv[:, :, 0:32], pv[:, 0:NH:2, :])
            nc.vector.tensor_copy(vv[:, :, 100:132], pv[:, 1:NH:2, :])
            V.append(vt)
        st["Vs"][ib] = V

    def s1_s(st, ib):
        """S^T = K^T Q (row-tiled) + exp, for batch ib of pair q."""
        b = st["b0"] + ib
        bo = ib * N
        Q, K = st["Q"], st["K"]
        PT = {}
        for g in range(CCH):
            for jh in range(2):
                pss = ps2.tile([P, 2, 512], F32, tag="ps2")
                for j2 in range(2):
                    j = 2 * jh + j2
                    for mc, (moff, mlen) in enumerate(MCHUNKS):
                        nc.tensor.matmul(
                            pss[:mlen, j2, mc * N:mc * N + N],
                            K[g][32 * j:32 * j + 32,
                                 bo + moff:bo + moff + mlen],
                            Q[g][32 * j:32 * j + 32, bo:bo + N],
                            start=True, stop=True,
                            tile_position=(32 * j, 0))
                ptt = ptp.tile([P, 2, 2 * N], BF16, tag="pt",
                               name=f"pt{b}_{g}_{jh}")
                nc.scalar.activation(ptt[:98, :, 0:2 * N],
                                     pss[:98, :, 0:2 * N], AF.Exp)
                PT[(g, jh)] = ptt
        st["PTs"][ib] = PT

    def s2_av(st, ib):
        """A@V + denominator extraction for batch ib of pair q."""
        b = st["b0"] + ib
        V, PT = st["Vs"][ib], st["PTs"][ib]
        oubs, recbs = [], []
        for pq in range(4):
            psav = psA.tile([P, 512], F32, tag="ps")
            for j in range(2):
                pr = 2 * pq + j
                nmm = 0
                for mc, (moff, mlen) in enumerate(MCHUNKS):
                    for e in range(2):
                        h = 2 * pr + e
                        g, jj = divmod(h, 4)
                        nc.tensor.matmul(
                            psav[:68, j * N:j * N + N],
                            V[mc][:mlen, h * 68:h * 68 + 68],
                            PT[(g, jj // 2)][:mlen, jj % 2,
                                             mc * N:mc * N + N],
                            start=(nmm == 0), stop=(nmm == 3))
                        nmm += 1
            den2 = rcp.tile([2, 2 * N], F32, tag="den",
                            name=f"den_{b}_{pq}")
            nc.vector.tensor_copy(den2[:], psav[64:66, :2 * N])
            oub = oup.tile([64, 2 * N], BF16, tag="oub",
                           name=f"oub_{b}_{pq}")
            nc.vector.tensor_copy(oub[:], psav[:64, :2 * N])
            rec2 = rcp.tile([2, 2 * N], F32, tag="rec",
                            name=f"rec_{b}_{pq}")
            nc.vector.reciprocal_approx_fast(rec2[:], den2[:])
            recb = rbp.tile([2, 2 * N], BF16, tag="recb",
                            name=f"recb_{b}_{pq}")
            nc.vector.tensor_copy(recb[:], rec2[:])
            oubs.append(oub)
            recbs.append(recb)
        st.setdefault("oubs2", {})[ib] = oubs
        st.setdefault("recbs2", {})[ib] = recbs

    def s2_fin(st, feed=None):
        """Softmax normalize (psb broadcast + OT) and proj+residual."""
        if feed is None:
            feed = lambda: None
        q, b0, xrs = st["q"], st["b0"], st["xrs"]
        OT = [otp.tile([P, 2, N], BF16, tag="ot", name=f"ot_{q}_{g_}")
              for g_ in range(CCH)]
        for ib in range(2):
            for pq in range(4):
                psb = psA.tile([P, 512], F32, tag="ps")
                nc.tensor.matmul(psb[:64, :2 * N], sel_sb[:, 0:64],
                                 st["recbs2"][ib][pq][:],
                                 start=True, stop=True)
                for j in range(2):
                    nc.vector.tensor_mul(
                        OT[pq][64 * j:64 * j + 64, ib, :],
                        st["oubs2"][ib][pq][:, j * N:j * N + N],
                        psb[:64, j * N:j * N + N])
            feed()
        for cc in range(CCH):
            psp = psA.tile([P, 512], F32, tag="ps")
            for g in range(CCH):
                nc.tensor.matmul(psp[:, :2 * N],
                                 pw_sb[g][:, cc * P:(cc + 1) * P],
                                 OT[g][:], start=(g == 0),
                                 stop=(g == CCH - 1))
            nc.vector.scalar_tensor_tensor(
                x2[cc][:, b0 * N:(b0 + 2) * N], psp[:, :2 * N],
                pb_sb[:, cc:cc + 1], xrs[cc][:], ALU.add, ALU.add)
            feed()

    # ================= ConvFFN (per half of 4 batches) =================
    def ffn_conv(hf):
        b0 = hf * HB
        xpvs = []
        for cc in range(CCH):
            xp = xps[cc]
            xpv = xp[:, :HB * PADW * PADW].rearrange(
                "p (b y x) -> p b y x", b=HB, y=PADW)
            nc.gpsimd.tensor_scalar(
                xpv[:, :, 3:3 + H, 3:3 + W],
                x2[cc][:, b0 * N:(b0 + HB) * N].rearrange(
                    "p (b y x) -> p b y x", b=HB, y=H),
                s1_sb[:, cc:cc + 1], t1_sb[:, cc:cc + 1], ALU.mult, ALU.add)
            xpvs.append(xpv)

        # conv: 49 taps x 16 concurrent 32x32 diag tiles. Block (cc, rb)
        # reads sbuf partitions 32*rb of xps[cc], writes psum partitions
        # 32*jj (jj=(rb+cc)%4) of conv psum tile cc -- per partition
        # range, the 4 row-groups land in 4 distinct banks.
        pA = [psA.tile([P, 512], F32, tag="ps", name=f"cps{hf}_{c}")
              for c in range(CCH)]
        pB = [ps2.tile([P, 2, 512], F32, tag="ps2", name=f"cps2_{hf}_{c}")
              for c in range(2)]
        psd = [[pA[c][:, :392] for c in range(CCH)],
               [pB[c // 2][:, c % 2, :392] for c in range(CCH)]]
        for t in range(NTAPS):
            dy, dx = divmod(t, 7)
            for cc in range(CCH):
                for rb in range(4):
                    jj = (rb + cc) % 4
                    for si in range(2):
                        mm = nc.tensor.matmul(
                            psd[si][cc][32 * jj:32 * jj + 32, :],
                            dwd_sb[32 * rb:32 * rb + 32,
                                   (t * CCH + cc) * 32:(t * CCH + cc) * 32 + 32],
                            xpvs[cc][32 * rb:32 * rb + 32,
                                     2 * si:2 * si + 2, dy:dy + H, dx:dx + W],
                            start=(t == 0), stop=(t == NTAPS - 1),
                            tile_position=(32 * rb, 32 * jj))
                        if si == 1:
                            # same 32x32 diag weights as the si=0 matmul
                            # just issued on this tile: skip the reload
                            mm.ldweights = False

        # evacuate conv psum -> sbuf bf16 (split across DVE and ACT)
        conv_sb = [[None] * CCH for _ in range(2)]
        for si in range(2):
            for cc in range(CCH):
                cv = cvp.tile([P, 392], BF16, tag="cv",
                              name=f"cv{hf}_{si}_{cc}")
                if cc < 2:
                    nc.vector.tensor_copy(cv[:], psd[si][cc])
                else:
                    nc.scalar.activation(cv[:], psd[si][cc], AF.Copy)
                conv_sb[si][cc] = cv
        return conv_sb

    # fc1 -> gelu -> fc2 -> residual out, per token slice of 392; emitted
    # as a list of closures so they can be interleaved into the attention
    # pairs' PE stall windows (A@V groups waiting on ScalarE exps)
    def ffn_fc_units(hf, conv_sb):
        units = []
        gehs = {}

        def fc1_unit(si, hc):
            def emit():
                s = 2 * hf + si
                if si not in gehs:
                    gehs[si] = [gep.tile([P, HCH // 2, 392], F8,
                                         tag=f"ge{si}_{h_}",
                                         name=f"ge{s}_{h_}")
                                for h_ in range(2)]
                geh = gehs[si]
                psf = psA.tile([P, 512], F32, tag="ps")
                for kt in range(CCH):
                    nc.tensor.matmul(
                        psf[:, :392], f1_sb[kt][:, hc * P:(hc + 1) * P],
                        conv_sb[si][kt][:],
                        start=(kt == 0), stop=(kt == CCH - 1))
                nc.scalar.activation(geh[hc // 8][:, hc % 8, :],
                                     psf[:, :392], AF.Gelu,
                                     bias=f1b_sb[:, hc:hc + 1])
            return emit

        def fc2_unit(si, cc):
            def emit():
                s = 2 * hf + si
                geh = gehs[si]
                psf2 = psA.tile([P, 512], F32, tag="ps")
                for kt in range(HCH):
                    nc.tensor.matmul(psf2[:, :392],
                                     f2_sb[kt][:, cc * P:(cc + 1) * P],
                                     geh[kt // 8][:, kt % 8, :],
                                     start=(kt == 0), stop=(kt == HCH - 1))
                ot = outp.tile([P, 392], F32, tag="oo")
                nc.vector.scalar_tensor_tensor(
                    ot[:], psf2[:, :392], s2_sb[:, cc:cc + 1],
                    x2[cc][:, s * 392:(s + 1) * 392], ALU.mult, ALU.add)
                nc.sync.dma_start(out[cc, :, 2 * s:2 * s + 2, :],
                                  ot.rearrange("p (b n) -> p b n", b=2))
            return emit

        for si in range(2):
            for hc in range(HCH):
                units.append(fc1_unit(si, hc))
        for si in range(2):
            for cc in range(CCH):
                units.append(fc2_unit(si, cc))
        return units

    def make_sink(units, per_call):
        state = {"i": 0}

        def sink():
            k = per_call
            while k > 0 and state["i"] < len(units):
                units[state["i"]]()
                state["i"] += 1
                k -= 1
        return sink, state

    # ---- 2-stage software pipeline over the 4 batch pairs ----
    # Window q emits, in PE-queue order:  QK(q+1) | AV(q,0) V(q+1,0)
    # S(q+1,0) | AV(q,1) V(q+1,1) S(q+1,1) | psb+proj(q).  Every chunk's
    # inputs are ready by the time the in-order PE queue reaches it:
    # AV(q) consumes exps produced one window earlier, and V(q+1)'s
    # evacuation into the shared vt scaffold happens after AV(q,ib) has
    # finished reading it.  x for pair q+2 is DMA-prefetched each window.
    sts = [dict(q=q, b0=2 * q) for q in range(4)]
    sts[0]["xrs"], sts[0]["xrb"] = x0
    sts[1]["xrs"], sts[1]["xrb"] = load_pair_x(1)

    st = sts[0]
    s1_qk(st)
    s1_s(st, 0)
    s1_v(st, 0)
    s1_s(st, 1)
    s1_v(st, 1)

    conv_sb0 = None
    for q in range(3):
        cur, nxt = sts[q], sts[q + 1]
        if q + 2 < 4:
            sts[q + 2]["xrs"], sts[q + 2]["xrb"] = load_pair_x(q + 2)
        s1_qk(nxt)
        s2_av(cur, 0)
        s1_v(nxt, 0)
        s1_s(nxt, 0)
        s2_av(cur, 1)
        s1_v(nxt, 1)
        s1_s(nxt, 1)
        s2_fin(cur)
        if q == 1 and not ATTN_ONLY:
            conv_sb0 = ffn_conv(0)
    if ATTN_ONLY:
        s2_av(sts[3], 0)
        s2_av(sts[3], 1)
        s2_fin(sts[3])
    else:
        units0 = ffn_fc_units(0, conv_sb0)
        sink0, fst0 = make_sink(units0, 2)
        for _ in range(3):
            sink0()
        s2_av(sts[3], 0)
        for _ in range(2):
            sink0()
        s2_av(sts[3], 1)
        s2_fin(sts[3], feed=sink0)
        while fst0["i"] < len(units0):
            units0[fst0["i"]]()
            fst0["i"] += 1
        conv_sb1 = ffn_conv(1)
        for u in ffn_fc_units(1, conv_sb1):
            u()

    ctx.close()


def _prep_inputs(x, bn_g, bn_b, bn_m, bn_v, qkv_w, proj_w, proj_b,
                 dw_w, fbn_g, fbn_b, fbn_m, fbn_v, fc1_w, fc1_b, fc2_w, fc2_b,
                 ls1, ls2):
    """Host-side folding of BN/layer-scale into weights; returns per-core in_maps."""
    f32 = np.float32
    bf = ml_dtypes.bfloat16
    x = np.asarray(x, f32)
    ls1v = np.asarray(ls1, f32).reshape(C)
    ls2v = np.asarray(ls2, f32).reshape(C)

    s1 = np.asarray(bn_g, f32) / np.sqrt(np.asarray(bn_v, f32) + EPS)
    t1 = np.asarray(bn_b, f32) - np.asarray(bn_m, f32) * s1

    qkv_w = np.asarray(qkv_w, f32)
    Wq, Wk, Wv = qkv_w[:C], qkv_w[C:2 * C], qkv_w[2 * C:]
    scale = D ** -0.5
    Wq_f = (Wq * s1[None, :]) * scale
    bq = (Wq @ t1) * scale
    Wk_f = Wk * s1[None, :]
    Wv_f = Wv * s1[None, :]
    bv = Wv @ t1

    proj_w = np.asarray(proj_w, f32)
    Wp_f = ls1v[:, None] * proj_w
    pb = ls1v * (np.asarray(proj_b, f32) + proj_w @ bv)
    # fc2 runs in fp8 with RAW weights (folding ls2 in would underflow
    # fp8); ls2 is applied per-partition in the output STT, and the
    # ls2-scaled fc2 bias is folded into the attention residual bias pb.

    fs = np.asarray(fbn_g, f32) / np.sqrt(np.asarray(fbn_v, f32) + EPS)
    ft = np.asarray(fbn_b, f32) - np.asarray(fbn_m, f32) * fs
    dww = np.asarray(dw_w, f32)[:, 0] * fs[:, None, None]      # [C,7,7]
    fc1_w = np.asarray(fc1_w, f32)
    fb1 = np.asarray(fc1_b, f32) + fc1_w @ ft
    fc2_w = np.asarray(fc2_w, f32)
    fb2_fold = ls2v * np.asarray(fc2_b, f32)
    pb = pb + fb2_fold

    # combined qkv weight, lhsT layout [CCH, 128, 3C]
    Wqkv = np.concatenate([Wq_f, Wk_f, Wv_f], axis=0)          # [3C, C]
    qkvw_t = np.ascontiguousarray(
        Wqkv.T.reshape(CCH, P, 3 * C)).astype(bf)
    projw_t = np.ascontiguousarray(Wp_f.T.reshape(CCH, P, C)).astype(bf)
    # conv-output channel permutation: conv psum tile cc partition
    # 32*jj+a holds channel cc*128 + ((jj-cc)%4)*32 + a; permute fc1's
    # input-channel rows to match.
    pidx = np.empty((CCH, P), np.int64)
    for cc in range(CCH):
        for jj in range(4):
            rb = (jj - cc) % 4
            pidx[cc, 32 * jj:32 * jj + 32] = (
                cc * P + rb * 32 + np.arange(32))
    fc1w_t = np.ascontiguousarray(fc1_w.T[pidx]).astype(bf)   # [CCH,P,HID]
    f8 = ml_dtypes.float8_e4m3
    fc2w_t = np.ascontiguousarray(fc2_w.T.reshape(HCH, P, C)).astype(f8)

    def colmajor(v, nch):
        return np.ascontiguousarray(v.reshape(nch, P).T).astype(f32)

    qb_t = colmajor(bq, CCH)
    pb_t = colmajor(pb, CCH)
    f1b_t = colmajor(fb1, HCH)
    f2b_t = colmajor(np.zeros(C, f32), CCH)
    s1_t = colmajor(s1, CCH)
    s2_t = colmajor(ls2v, CCH)
    t1_t = colmajor(t1, CCH)

    # dwdiag [128, NTAPS*CCH*32]: for tap t, chunk cc, block rb: 32x32
    # diag(dww[cc*128+32*rb : +32, t]) at partitions [32*rb, +32),
    # cols [(t*CCH+cc)*32, +32)
    wflat = dww.reshape(C, NTAPS)                               # [C, 49]
    dwd = np.zeros((4, 32, NTAPS, CCH, 32), f32)                # rb,a,t,cc,b
    a32 = np.arange(32)
    for rb in range(4):
        for cc in range(CCH):
            dwd[rb, a32, :, cc, a32] = wflat[cc * P + rb * 32 + a32]
    dwd = np.ascontiguousarray(
        dwd.reshape(P, NTAPS * CCH * 32)).astype(bf)

    # sel2[e, 32e+d] = 1: broadcasts rec row e to oub rows 32e+0:32
    sel = np.zeros((2, P), f32)
    sel[0, 0:32] = 1.0
    sel[1, 32:64] = 1.0
    sel = sel.astype(bf)

    # x shards: [CCH, 128, BPC, N]
    xr = x.reshape(NCORES, BPC, C, N)
    shared = dict(qkvw=qkvw_t, qb=qb_t, projw=projw_t, projb=pb_t,
                  fc1w=fc1w_t, fc1b=f1b_t, fc2w=fc2w_t, fc2b=f2b_t,
                  s1d=s1_t, s2d=s2_t, t1d=t1_t, dwdiag=dwd, selw=sel)
    in_maps = []
    for c in range(NCORES):
        xc = np.ascontiguousarray(
            xr[c].reshape(BPC, CCH, P, N).transpose(1, 2, 0, 3))
        in_maps.append(dict(shared, xin=xc))
    return in_maps


def _get_nc():
    if "nc" not in _CACHE:
        _CACHE["nc"] = _build_nc()
    return _CACHE["nc"]


def _gather(results):
    outs = []
    for c in range(NCORES):
        oc = results[c]["out"]              # [CCH, P, BPC, N]
        outs.append(oc.transpose(2, 0, 1, 3).reshape(BPC, C, H, W))
    return np.concatenate(outs, axis=0).astype(np.float32)


def kernel(**inputs):
    nc = _get_nc()
    in_maps = _prep_inputs(**inputs)
    res = run_bass_kernel_spmd(nc, in_maps, list(range(NCORES)))
    return _gather(res.results)


if __name__ == "__main__":
    rng = np.random.default_rng(0)
    ins = dict(
        x=rng.normal(size=(B, C, H, W)).astype(np.float32),
        bn_g=1.0 + 0.1 * rng.normal(size=C).astype(np.float32),
        bn_b=0.1 * rng.normal(size=C).astype(np.float32),
        bn_m=0.1 * rng.normal(size=C).astype(np.float32),
        bn_v=rng.uniform(0.5, 1.5, size=C).astype(np.float32),
        qkv_w=0.02 * rng.normal(size=(3 * C, C)).astype(np.float32),
        proj_w=0.02 * rng.normal(size=(C, C)).astype(np.float32),
        proj_b=np.zeros(C, np.float32),
        dw_w=0.02 * rng.normal(size=(C, 1, 7, 7)).astype(np.float32),
        fbn_g=1.0 + 0.1 * rng.normal(size=C).astype(np.float32),
        fbn_b=0.1 * rng.normal(size=C).astype(np.float32),
        fbn_m=0.1 * rng.normal(size=C).astype(np.float32),
        fbn_v=rng.uniform(0.5, 1.5, size=C).astype(np.float32),
        fc1_w=0.02 * rng.normal(size=(HID, C)).astype(np.float32),
        fc1_b=np.zeros(HID, np.float32),
        fc2_w=0.02 * rng.normal(size=(C, HID)).astype(np.float32),
        fc2_b=np.zeros(C, np.float32),
        ls1=1e-5 * np.ones((C, 1, 1), np.float32),
        ls2=1e-5 * np.ones((C, 1, 1), np.float32),
    )
    o = kernel(**ins)
    print("out", o.shape, o.dtype, float(np.abs(o).max()))

